# revision 16
# baseline (speedup 1.0000x reference)
"""CondConv (MoE routed conv) Trainium2 Bass kernel — v3.

Sharding: core c -> samples [4*(c//2), 4*(c//2)+4), cout half ot = c%2.
Per-core DMA: x 6.65MB + its cout-half of weights 4.72MB, all bf16.

Engine plan (per core):
  - PE: 3x3 conv as 18 accumulating bf16 matmuls per [128 x 8 x 56] PSUM
    block, phase-A (it0 taps) / phase-B (it1 taps) per sample so the it1
    combined weights are never needed early. PE also combines cw(0,0)
    via diag-scaled matmuls during its DMA-wait head, paced by slab
    arrivals. bf16 junk matmuls (memset operands) hold the clock up.
  - DVE: the other 7 cw tiles as tensor_scalar (2.3x) + tensor_tensor
    (2x) ping-pong chains — STT has no fast uop on HW (1x), TS/TT do.
  - Scalar: GAP reduces via in-place Copy+accum, sigmoids (table
    preloaded at t=0), diag tiles, BN+SiLU epilogues (bf16 out).
  - Sync ring: all input DMA + y output DMA. GpSimd ring: small loads.
"""

import sys

sys.path.insert(0, "/opt/trn_rl_repo")

import ml_dtypes
import numpy as np

import concourse.bass as bass  # noqa: F401
import concourse.mybir as mybir
import concourse.tile as tile
from concourse import bacc
from concourse.bass_utils import run_bass_kernel_spmd

F32 = mybir.dt.float32
BF16 = mybir.dt.bfloat16
AF = mybir.ActivationFunctionType
ALU = mybir.AluOpType
NPBF16 = ml_dtypes.bfloat16

B, CIN, H, W = 16, 256, 56, 56
E, COUT, KS = 8, 256, 3
NCORES = 8
SPC = 4
IT = CIN // 128
OT = COUT // 128
KHKW = KS * KS
HB = 8
WP = W + 2
PIX = H * W
BN_EPS = 1e-5
SLAB = KHKW * 128  # 1152
CHUNK = 384
NCH = SLAB // CHUNK
NPA = 7
JUNK_A = 34
JUNK_B = 48
HHALF = H // 2

_PROGRAM_CACHE = {}


def _build_program():
    nc = bacc.Bacc("TRN2", target_bir_lowering=False, debug=False)

    x_d = nc.dram_tensor("x", [SPC, IT, 128, H, WP], BF16, kind="ExternalInput")
    wt_d = nc.dram_tensor("wt", [E, IT, 128, SLAB], BF16, kind="ExternalInput")
    rwt_d = nc.dram_tensor("rwt", [IT, 128, E], F32, kind="ExternalInput")
    rb_d = nc.dram_tensor("rb", [1, E], F32, kind="ExternalInput")
    bns_d = nc.dram_tensor("bns", [128, 1], F32, kind="ExternalInput")
    bnb_d = nc.dram_tensor("bnb", [128, 1], F32, kind="ExternalInput")
    y_d = nc.dram_tensor("y", [SPC, 128, H, W], BF16, kind="ExternalOutput")

    with tile.TileContext(nc) as tc:
        with (
            tc.tile_pool(name="xp", bufs=1) as xp,
            tc.tile_pool(name="cwp", bufs=1) as cwp,
            tc.tile_pool(name="wtp", bufs=1) as wtp,
            tc.tile_pool(name="outp", bufs=6) as outp,
            tc.tile_pool(name="smal", bufs=1) as smal,
            tc.tile_pool(name="psc", bufs=NPA, space="PSUM") as psc,
            tc.tile_pool(name="pss", bufs=1, space="PSUM") as pss,
        ):
            # ---- junk operands + sigmoid table preload (no DMA deps) ----
            jx = smal.tile([128, 448], BF16, tag="jx")
            nc.vector.memset(jx[:], 0.25)
            ones_sb = smal.tile([1, 128], F32, tag="ones")
            nc.vector.memset(ones_sb[:], 1.0)
            sig_dummy = smal.tile([1, 8], BF16, tag="sigd")
            nc.scalar.activation(sig_dummy[:], jx[0:1, 0:8], AF.Sigmoid)

            x_sb = {}

            def load_x(s, halves):
                for it in range(IT):
                    t = xp.tile(
                        [128, H, WP], BF16, tag=f"x_{s}_{it}", name=f"x_{s}_{it}"
                    )
                    if halves:
                        nc.sync.dma_start(t[:, 0:HHALF, :], x_d[s, it, :, 0:HHALF, :])
                        nc.sync.dma_start(t[:, HHALF:H, :], x_d[s, it, :, HHALF:H, :])
                    else:
                        nc.sync.dma_start(t[:], x_d[s, it])
                    x_sb[s, it] = t

            slab_tiles = {}

            def load_slabs(it):
                for e in range(E):
                    wt_t = wtp.tile(
                        [128, SLAB], BF16, tag=f"wt{it}{e}", name=f"wt{it}{e}"
                    )
                    nc.sync.dma_start(wt_t[:], wt_d[e, it])
                    slab_tiles[it, e] = wt_t

            load_x(0, halves=True)
            load_slabs(0)
            load_slabs(1)
            load_x(1, halves=False)
            load_x(2, halves=False)
            load_x(3, halves=False)

            # small loads on the gpsimd ring
            rwt_sb = []
            for it in range(IT):
                t = smal.tile([128, E], F32, tag=f"rwt{it}", name=f"rwt{it}")
                nc.gpsimd.dma_start(t[:], rwt_d[it])
                rwt_sb.append(t)
            rb_sb = smal.tile([1, E], F32, tag="rb")
            nc.gpsimd.dma_start(rb_sb[:], rb_d[:])
            bns_sb = smal.tile([128, 1], F32, tag="bns")
            nc.gpsimd.dma_start(bns_sb[:], bns_d[:])
            bnb_sb = smal.tile([128, 1], F32, tag="bnb")
            nc.gpsimd.dma_start(bnb_sb[:], bnb_d[:])

            def warmup(n):
                # rotate PSUM banks: same-bank WAW serializes at ~530ns/mm
                for _ in range(n):
                    jps = psc.tile([128, HB, W], F32, tag="ps", name="jps")
                    flat = jps[:].rearrange("p a b -> p (a b)")
                    nc.tensor.matmul(
                        flat[:, 0:448], jx[:, 0:128], jx[:], start=True, stop=True
                    )

            # ---- routing ----
            pooled = {}  # (s, it, half) -> [128,1] partial sums
            rrow = {}
            r_bcast = {}

            def reduce_dve_half(s, it, h):
                p = smal.tile([128, 1], F32, tag=f"po{s}{it}{h}", name=f"po{s}{it}{h}")
                sl = x_sb[s, it][:, h * HHALF : (h + 1) * HHALF, :]
                nc.vector.reduce_sum(p[:], sl, axis=mybir.AxisListType.XY)
                pooled[s, it, h] = p

            def reduce_act_half(s, it, h):
                p = smal.tile([128, 1], F32, tag=f"po{s}{it}{h}", name=f"po{s}{it}{h}")
                sl = x_sb[s, it][:, h * HHALF : (h + 1) * HHALF, :]
                nc.scalar.activation(sl, sl, AF.Copy, accum_out=p[:])
                pooled[s, it, h] = p

            def reduce_act_full(s, it):
                p = smal.tile([128, 1], F32, tag=f"po{s}{it}0", name=f"po{s}{it}0")
                flat = x_sb[s, it][:].rearrange("p a b -> p (a b)")
                nc.scalar.activation(flat, flat, AF.Copy, accum_out=p[:])
                pooled[s, it, 0] = p

            def reduce_dve_full(s, it):
                p = smal.tile([128, 1], F32, tag=f"po{s}{it}0", name=f"po{s}{it}0")
                nc.vector.reduce_sum(
                    p[:],
                    x_sb[s, it][:].rearrange("p a b -> p (a b)"),
                    axis=mybir.AxisListType.X,
                )
                pooled[s, it, 0] = p

            def routing_logits_pe(s):
                lg_ps = pss.tile([1, E], F32, tag="rps", name=f"lgps{s}")
                parts = [k for k in pooled if k[0] == s]
                for i, (s_, it, h) in enumerate(parts):
                    nc.tensor.matmul(
                        lg_ps[:], pooled[s_, it, h][:], rwt_sb[it][:],
                        start=(i == 0), stop=(i == len(parts) - 1),
                    )
                return lg_ps

            def routing_tail(s, lg_ps):
                # z on DVE, broadcast z via PE, sigmoid straight to [128,E]
                zr = smal.tile([1, E], F32, tag=f"z{s}", name=f"z{s}")
                nc.vector.scalar_tensor_tensor(
                    zr[:], lg_ps[:], 1.0 / PIX, rb_sb[:], ALU.mult, ALU.add
                )
                zb_ps = pss.tile([128, E], F32, tag="rps", name=f"zbps{s}")
                nc.tensor.matmul(zb_ps[:], ones_sb[:], zr[:], start=True, stop=True)
                rbc = smal.tile([128, E], F32, tag=f"rbc{s}", name=f"rbc{s}")
                nc.scalar.activation(rbc[:], zb_ps[:], AF.Sigmoid)
                r_bcast[s] = rbc

            cw_r = {
                (s, it): cwp.tile(
                    [128, SLAB], BF16, tag=f"cwr_{s}_{it}", name=f"cwr_{s}_{it}"
                )
                for s in range(SPC)
                for it in range(IT)
            }
            acc_a = cwp.tile([128, SLAB], BF16, tag="acc_a")
            acc_b = cwp.tile([128, SLAB], BF16, tag="acc_b")
            tmp_t = cwp.tile([128, SLAB], BF16, tag="tmp")
            u_a = cwp.tile([128, SLAB], BF16, tag="u_a")
            u_b = cwp.tile([128, SLAB], BF16, tag="u_b")

            def combine_dve(s, it, act_assist=False):
                # TS (2.3x) + TT (2x) ping-pong chain; STT would be 1x.
                # With act_assist, odd-e scales run on the Scalar engine in
                # parallel so the DVE-serial latency is ~30% shorter.
                dst = cw_r[s, it]
                accs = [acc_a, acc_b]
                us = [u_a, u_b]
                cur = None
                for e in range(E):
                    wt_t = slab_tiles[it, e]
                    sc = r_bcast[s][:, e : e + 1]
                    if e == 0:
                        cur = accs[0]
                        nc.vector.tensor_scalar_mul(cur[:], wt_t[:], sc)
                        continue
                    if act_assist and (e % 2 == 1):
                        u = us[(e // 2) % 2]
                        nc.scalar.activation(u[:], wt_t[:], AF.Copy, scale=sc)
                    else:
                        u = tmp_t
                        nc.vector.tensor_scalar_mul(u[:], wt_t[:], sc)
                    nxt = dst if e == E - 1 else accs[e % 2]
                    nc.vector.tensor_tensor(nxt[:], cur[:], u[:], ALU.add)
                    cur = nxt

            hblocks = [(h0, min(HB, H - h0)) for h0 in range(0, H, HB)]
            taps = [(0, 0)] + [
                (dh, dw) for dh in (-1, 0, 1) for dw in (-1, 0, 1) if (dh, dw) != (0, 0)
            ]

            def block_total(h0, nh):
                return IT * sum(
                    1 for dh, dw in taps if min(h0 + nh, H - dh) > max(h0, -dh)
                )

            def conv_block_taps(s, h0, nh, ps_t, its, n_mm, total):
                for it in its:
                    for dh, dw in taps:
                        khkw = (dh + 1) * 3 + (dw + 1)
                        ho_s = max(h0, -dh)
                        ho_e = min(h0 + nh, H - dh)
                        if ho_e <= ho_s:
                            continue
                        nhh = ho_e - ho_s
                        hi_s = ho_s + dh
                        off = khkw * 128
                        lhsT = cw_r[s, it][:, off : off + 128]
                        rhs = x_sb[s, it][:, hi_s : hi_s + nhh, 1 + dw : 1 + dw + W]
                        out = ps_t[:, ho_s - h0 : ho_s - h0 + nhh, 0:W]
                        nc.tensor.matmul(
                            out, lhsT, rhs,
                            start=(n_mm == 0), stop=(n_mm == total - 1),
                        )
                        n_mm += 1
                return n_mm

            def conv_epilogue(s, h0, nh, ps_t):
                o_t = outp.tile([128, HB, W], BF16, tag="out", name="o_t")
                nc.scalar.activation(
                    o_t[:, :nh, :], ps_t[:, :nh, :], AF.Silu,
                    bias=bnb_sb[:], scale=bns_sb[:],
                )
                nc.sync.dma_start(y_d[s, :, h0 : h0 + nh, :], o_t[:, :nh, :])

            # ================= emission schedule =================
            # s0 routing: split-half reduces on DVE + ACT in parallel.
            # Junk is split around the routing PE ops so they aren't
            # queue-blocked behind it (4-deep wait queue parks the bcast).
            reduce_dve_half(0, 0, 0)
            reduce_act_half(0, 0, 1)
            reduce_dve_half(0, 1, 0)
            reduce_act_half(0, 1, 1)
            warmup(JUNK_A)
            lg0 = routing_logits_pe(0)
            routing_tail(0, lg0)
            warmup(JUNK_B)

            combine_dve(0, 0, act_assist=True)
            combine_dve(0, 1)

            # phase-A s0
            pa = []
            for h0, nh in hblocks:
                ps_t = psc.tile([128, HB, W], F32, tag="ps", name="ps")
                total = block_total(h0, nh)
                n_mm = conv_block_taps(0, h0, nh, ps_t, [0], 0, total)
                pa.append((h0, nh, ps_t, n_mm, total))

            # s1 reduces (x1 lands ~32us): DVE + ACT in parallel
            reduce_dve_full(1, 0)
            reduce_act_full(1, 1)

            # phase-B s0: s1 routing after block 0; s2 GAP reduces as
            # half-tile pieces spread across blocks so the Scalar queue
            # never delays a PSUM-bank-freeing epilogue by more than
            # ~1.5us.
            for bi, (h0, nh, ps_t, n_mm, total) in enumerate(pa):
                n_mm = conv_block_taps(0, h0, nh, ps_t, [1], n_mm, total)
                assert n_mm == total
                conv_epilogue(0, h0, nh, ps_t)
                if bi == 0:
                    lg1 = routing_logits_pe(1)
                    routing_tail(1, lg1)
                    combine_dve(1, 0, act_assist=True)
                    combine_dve(1, 1)
                if bi in (1, 2, 3, 4):
                    reduce_act_half(2, (bi - 1) // 2, (bi - 1) % 2)

            def conv_sample_phased(s, weave_a=None, weave_b=None):
                weave_a = weave_a or {}
                weave_b = weave_b or {}
                pa_s = []
                for bi, (h0, nh) in enumerate(hblocks):
                    ps_t = psc.tile([128, HB, W], F32, tag="ps", name="ps")
                    total = block_total(h0, nh)
                    n_mm = conv_block_taps(s, h0, nh, ps_t, [0], 0, total)
                    pa_s.append((h0, nh, ps_t, n_mm, total))
                    if bi in weave_a:
                        weave_a[bi]()
                for bi, (h0, nh, ps_t, n_mm, total) in enumerate(pa_s):
                    n_mm = conv_block_taps(s, h0, nh, ps_t, [1], n_mm, total)
                    assert n_mm == total
                    conv_epilogue(s, h0, nh, ps_t)
                    if bi in weave_b:
                        weave_b[bi]()

            def weave_s2_routing():
                lg2 = routing_logits_pe(2)
                routing_tail(2, lg2)
                combine_dve(2, 0)
                combine_dve(2, 1)

            def weave_s3_routing():
                lg3 = routing_logits_pe(3)
                routing_tail(3, lg3)
                combine_dve(3, 0)
                combine_dve(3, 1)

            conv_sample_phased(
                1,
                weave_a={1: weave_s2_routing,
                         2: lambda: reduce_act_half(3, 0, 0),
                         4: lambda: reduce_act_half(3, 0, 1)},
                weave_b={0: lambda: reduce_act_half(3, 1, 0),
                         2: lambda: reduce_act_half(3, 1, 1)},
            )
            conv_sample_phased(2, weave_a={1: weave_s3_routing})
            conv_sample_phased(3)

    nc.compile()
    return nc


def _get_program():
    if "nc" not in _PROGRAM_CACHE:
        _PROGRAM_CACHE["nc"] = _build_program()
    return _PROGRAM_CACHE["nc"]


def kernel(x, routing_w, routing_b, kernel_weights, bn_gamma, bn_beta, bn_mean, bn_var,
           _trace=False, _trace_kwargs=None):
    x = np.asarray(x, dtype=np.float32)
    routing_w = np.asarray(routing_w, dtype=np.float32)
    routing_b = np.asarray(routing_b, dtype=np.float32)
    kernel_weights = np.asarray(kernel_weights, dtype=np.float32)
    bn_gamma = np.asarray(bn_gamma, dtype=np.float32)
    bn_beta = np.asarray(bn_beta, dtype=np.float32)
    bn_mean = np.asarray(bn_mean, dtype=np.float32)
    bn_var = np.asarray(bn_var, dtype=np.float32)

    kwb = kernel_weights.astype(NPBF16)
    kw7 = kwb.reshape(E, OT, 128, IT, 128, KS, KS)
    wt_host = [
        np.ascontiguousarray(kw7[:, ot].transpose(0, 2, 3, 4, 5, 1)).reshape(
            E, IT, 128, SLAB
        )
        for ot in range(OT)
    ]
    rwt_host = np.ascontiguousarray(routing_w.T).reshape(IT, 128, E)
    rb_host = np.ascontiguousarray(routing_b).reshape(1, E)
    inv = bn_gamma / np.sqrt(bn_var + BN_EPS)
    bnb_full = bn_beta - bn_mean * inv
    bns_host = [
        np.ascontiguousarray(inv[ot * 128 : (ot + 1) * 128]).reshape(128, 1)
        for ot in range(OT)
    ]
    bnb_host = [
        np.ascontiguousarray(bnb_full[ot * 128 : (ot + 1) * 128]).reshape(128, 1)
        for ot in range(OT)
    ]

    x_pad = np.zeros((B, CIN, H, WP), dtype=NPBF16)
    x_pad[:, :, :, 1 : 1 + W] = x.astype(NPBF16)
    x_host = [
        np.ascontiguousarray(
            x_pad[g * SPC : (g + 1) * SPC].reshape(SPC, IT, 128, H, WP)
        )
        for g in range(B // SPC)
    ]

    in_maps = []
    for c in range(NCORES):
        ot = c % 2
        g = c // 2
        in_maps.append(
            {
                "x": x_host[g],
                "wt": wt_host[ot],
                "rwt": rwt_host,
                "rb": rb_host,
                "bns": bns_host[ot],
                "bnb": bnb_host[ot],
            }
        )

    nc = _get_program()
    res = run_bass_kernel_spmd(
        nc, in_maps, core_ids=list(range(NCORES)),
        trace=_trace, **(_trace_kwargs or {}),
    )
    _PROGRAM_CACHE["last_result"] = res

    out = np.empty((B, COUT, H, W), dtype=np.float32)
    for c in range(NCORES):
        ot = c % 2
        g = c // 2
        yg = res.results[c]["y"]
        out[g * SPC : (g + 1) * SPC, ot * 128 : (ot + 1) * 128] = np.asarray(
            yg
        ).astype(np.float32)
    return out
